# revision 9
# baseline (speedup 1.0000x reference)
"""3-layer GAT (graph attention network) on 8 Trainium2 NeuronCores.

Problem: N=4096 nodes, dense adjacency [N,N], 3 GAT layers
  (128 -> 4x64, 256 -> 4x64, 256 -> 1x64), LeakyReLU(0.2) attention,
  masked softmax, ELU between layers.

Sharding: 1D row partition of the attention matrix. Each core owns
IB=512 rows i (queries). Scores/softmax/aggregation for those rows are
computed locally in transposed layout P[j, i] (j on partitions) so the
aggregation matmul contracts j on the partition axis with no transposes.
Between layers, per-core feature blocks h (bf16) and attention score
terms er (f32) are all-gathered.

Inner loop per (head, j-tile):
  e = er[j] (+) el[i]        -- one K=2 matmul into PSUM (outer-product add)
  s = LeakyReLU(e)           -- ScalarE, alpha=0.2
  p = exp(s)                 -- ScalarE, f32 -> bf16 (no max-shift needed:
                                scores are O(20), exp fits f32/bf16 range)
  p *= adjT mask             -- VectorE, bf16, batched over 8 j-tiles
  oT[o,i] += [h|1]^T @ p     -- TensorE, accumulating; row 64 = softmax denom

kernel(**inputs) takes the full unsharded inputs and returns the full
[4096, 64] output.
"""

import numpy as np
import ml_dtypes

import concourse.bass as bass
import concourse.mybir as mybir
import concourse.tile as tile
from concourse import bacc
from concourse.bass_utils import run_bass_kernel_spmd

F32 = mybir.dt.float32
BF16 = mybir.dt.bfloat16
AF = mybir.ActivationFunctionType
ALU = mybir.AluOpType

NC = 8          # cores
N = 4096        # nodes
NT = N // 128   # 32 j-tiles
GJ = 4          # j-tiles per score group (score batch = [128, GJ*IB] scores)
IB = N // NC    # 512 rows per core
H = 4           # heads (layers 0,1)
O = 64          # per-head output dim
D0 = 128        # layer-0 input dim
D1 = H * O      # 256, layer-1/2 input dim
ALPHA = 0.2

_CACHE = {}


def _dma(nc, out, in_):
    nc.sync.dma_start(out=out, in_=in_)


def _build():
    nc = bacc.Bacc(None, target_bir_lowering=False, num_devices=NC)

    xTf = nc.dram_tensor("xTf", [D0, N], F32, kind="ExternalInput")
    xT0 = nc.dram_tensor("xT0", [D0, IB], F32, kind="ExternalInput")
    maskT = nc.dram_tensor("maskT", [NT, 128, IB], BF16, kind="ExternalInput")
    w0 = nc.dram_tensor("w0", [H, D0, O], F32, kind="ExternalInput")
    w1 = nc.dram_tensor("w1", [H, D1, O], F32, kind="ExternalInput")
    w2 = nc.dram_tensor("w2", [1, D1, O], F32, kind="ExternalInput")
    wlr0 = nc.dram_tensor("wlr0", [D0, 2 * H], F32, kind="ExternalInput")
    wlr1 = nc.dram_tensor("wlr1", [D1, 2 * H], F32, kind="ExternalInput")
    wlr2 = nc.dram_tensor("wlr2", [D1, 2], F32, kind="ExternalInput")
    y = nc.dram_tensor("y", [O, IB], F32, kind="ExternalOutput")

    rg = [list(range(NC))]

    with tile.TileContext(nc) as tc:
        with (
            tc.tile_pool(name="const", bufs=1) as cpool,
            tc.tile_pool(name="work", bufs=2) as wpool,
            tc.tile_pool(name="psum", bufs=2, space="PSUM") as pp,
            tc.tile_pool(name="dram", bufs=1, space="DRAM") as dpool,
        ):
            # ---------- constants / resident tiles ----------
            mask_sb = cpool.tile([128, NT * IB], BF16)
            mv = mask_sb[:].rearrange("p (t i) -> p t i", t=NT)
            for g in range(4):
                _dma(nc, mv[:, g * 8:(g + 1) * 8, :],
                     maskT[g * 8:(g + 1) * 8].rearrange("t p i -> p t i"))

            xTf_sb = cpool.tile([D0, N], F32)
            _dma(nc, xTf_sb[:], xTf[:])
            xT0_sb = cpool.tile([D0, IB], F32)
            _dma(nc, xT0_sb[:], xT0[:])

            w0_sb = cpool.tile([D0, H * O], F32)
            _dma(nc, w0_sb[:].rearrange("d (h o) -> d h o", h=H),
                 w0[:].rearrange("h d o -> d h o"))
            w1_sb = cpool.tile([128, 2 * H * O], F32)  # [kc] chunks side by side
            w1v = w1_sb[:].rearrange("d (k h o) -> d k h o", k=2, h=H)
            w1s = w1[:].rearrange("h (k d) o -> k d h o", k=2)
            for kc in range(2):
                _dma(nc, w1v[:, kc], w1s[kc])
            w2_sb = cpool.tile([128, 2 * O], F32)
            w2v = w2_sb[:].rearrange("d (k h o) -> d k h o", k=2, h=1)
            w2s = w2[:].rearrange("h (k d) o -> k d h o", k=2)
            for kc in range(2):
                _dma(nc, w2v[:, kc], w2s[kc])

            wlr0_sb = cpool.tile([D0, 2 * H], F32)
            _dma(nc, wlr0_sb[:], wlr0[:])
            wlr1_sb = cpool.tile([128, 2 * 2 * H], F32)
            wlr1v = wlr1_sb[:].rearrange("d (k c) -> d k c", k=2)
            _dma(nc, wlr1v, wlr1[:].rearrange("(k d) c -> d k c", k=2))
            wlr2_sb = cpool.tile([128, 2 * 2], F32)
            wlr2v = wlr2_sb[:].rearrange("d (k c) -> d k c", k=2)
            _dma(nc, wlr2v, wlr2[:].rearrange("(k d) c -> d k c", k=2))

            ones_c = cpool.tile([1, IB], F32)
            nc.vector.memset(ones_c[:], 1.0)
            alpha_c = cpool.tile([128, 1], F32)
            nc.vector.memset(alpha_c[:], ALPHA)

            # ---------- DRAM bounce buffers for collectives ----------
            gh1_in = dpool.tile([IB, D1], BF16)
            gh1 = dpool.tile([N, D1], BF16, addr_space="Shared")
            ger1_in = dpool.tile([H, IB], F32)
            ger1 = dpool.tile([NC * H, IB], F32, addr_space="Shared")
            gh2_in = dpool.tile([IB, O], BF16)
            gh2 = dpool.tile([N, O], BF16, addr_space="Shared")
            ger2_in = dpool.tile([1, IB], F32)
            ger2 = dpool.tile([NC, IB], F32, addr_space="Shared")

            # ================= layer 0 prep =================
            # full h0 (redundantly per core) -> h_all0 [128, NT*(H*65)]
            h_all0 = wpool.tile([128, NT * H * 65], BF16, tag="h_all", bufs=1)
            nc.vector.memset(h_all0[:], 1.0)
            for jt in range(NT):
                ph = pp.tile([128, H * O], F32, tag="work", name=f"ph0_{jt}")
                for h in range(H):
                    nc.tensor.matmul(
                        ph[:, h * O:(h + 1) * O],
                        xTf_sb[:, jt * 128:(jt + 1) * 128],
                        w0_sb[:, h * O:(h + 1) * O],
                    )
                dst = h_all0[:, jt * H * 65:(jt + 1) * H * 65]
                dst = dst.rearrange("p (h c) -> p h c", h=H)[:, :, 0:O]
                nc.vector.tensor_copy(dst, ph[:].rearrange("p (h o) -> p h o", h=H))

            # er0 (all j) / el0 (local i), layer-0 score terms
            er_stage0 = wpool.tile([2 * H, N], F32, tag="er_stage", bufs=1)
            for c in range(NC):
                pe0 = pp.tile([2 * H, IB], F32, tag="work", name=f"pe0_{c}")
                nc.tensor.matmul(pe0[:], wlr0_sb[:], xTf_sb[:, c * IB:(c + 1) * IB])
                nc.vector.tensor_copy(er_stage0[:, c * IB:(c + 1) * IB], pe0[:])
            pl0 = pp.tile([2 * H, IB], F32, tag="work")
            nc.tensor.matmul(pl0[:], wlr0_sb[:], xT0_sb[:])
            el_stage0 = wpool.tile([2 * H, IB], F32, tag="elr_st", bufs=2)
            nc.vector.tensor_copy(el_stage0[:], pl0[:])

            def attention(nheads, h_all, lname, load_er, load_el):
                """Row-block attention for one layer.

                load_er(erp, h): fill erp[0:1, :] ([1, N] f32) with head h's er.
                load_el(elp, h): fill elp[1:2, :] ([1, IB] f32) with head h's el.
                Returns PSUM accumulators oT[h] [65, IB] (row 64 = denominator).
                """
                oT = [
                    pp.tile([65, IB], F32, tag=f"oT{h}", bufs=1, name=f"oT_{lname}_{h}")
                    for h in range(nheads)
                ]
                for h in range(nheads):
                    erp = wpool.tile([2, N], F32, tag="erp", bufs=2,
                                     name=f"erp_{lname}_{h}")
                    nc.vector.memset(erp[0:1, :], 1.0)
                    load_er(erp, h)
                    elp = wpool.tile([2, IB], F32, tag="elp", bufs=2,
                                     name=f"elp_{lname}_{h}")
                    nc.vector.memset(elp[:], 1.0)
                    load_el(elp, h)
                    for g in range(NT // GJ):
                        s8 = wpool.tile([128, GJ * IB], F32, tag="s8", bufs=2,
                                        name=f"s_{lname}_{h}_{g}")
                        for jp in range(GJ // 2):
                            jt = g * GJ + 2 * jp
                            pe = pp.tile([128, 2 * IB], F32, tag="work",
                                         name=f"pe_{lname}_{h}_{g}_{jp}")
                            for k in range(2):
                                nc.tensor.matmul(
                                    pe[:, k * IB:(k + 1) * IB],
                                    erp[0:2, (jt + k) * 128:(jt + k + 1) * 128],
                                    elp[0:2, :],
                                )
                            nc.scalar.activation(
                                s8[:, jp * 2 * IB:(jp + 1) * 2 * IB],
                                pe[:], AF.Prelu, alpha=alpha_c[:])
                        p8 = wpool.tile([128, GJ * IB], BF16, tag="p8", bufs=2,
                                        name=f"p_{lname}_{h}_{g}")
                        nc.scalar.activation(p8[:], s8[:], AF.Exp)
                        nc.vector.tensor_mul(
                            p8[:], p8[:],
                            mask_sb[:, g * GJ * IB:(g + 1) * GJ * IB])
                        for jj in range(GJ):
                            jt = g * GJ + jj
                            nc.tensor.matmul(
                                oT[h][:],
                                h_all[:, (jt * nheads + h) * 65:
                                      (jt * nheads + h) * 65 + 65],
                                p8[:, jj * IB:(jj + 1) * IB],
                                start=(jt == 0), stop=(jt == NT - 1),
                            )
                return oT

            def normalize(oTh, h, lname):
                """softmax-normalize one head: returns SBUF [64, IB] f32 tile."""
                recip = wpool.tile([1, IB], F32, tag="recip", bufs=2,
                                   name=f"rc_{lname}_{h}")
                nc.vector.reciprocal(recip[:], oTh[64:65, :])
                prb = pp.tile([O, IB], F32, tag="work", name=f"prb_{lname}_{h}")
                nc.tensor.matmul(prb[:], ones_c[0:1, 0:O], recip[:])
                rb = wpool.tile([O, IB], F32, tag="rb", bufs=2, name=f"rb_{lname}_{h}")
                nc.scalar.copy(rb[:], prb[:])
                z = wpool.tile([O, IB], F32, tag="z", bufs=2, name=f"z_{lname}_{h}")
                nc.vector.tensor_mul(z[:], oTh[0:64, :], rb[:])
                return z

            def transition(oT, nheads, w_sb, wlr_sb, next_heads, gh_in, gh,
                           ger_in, ger, h_all_n, lname):
                """ELU -> xTn; next-layer h/el/er; all-gathers; next-layer tiles.

                Returns (load_er, load_el) closures for the next layer.
                """
                xTn = [wpool.tile([128, IB], F32, tag=f"xTn{k}", bufs=2,
                                  name=f"xTn_{lname}_{k}") for k in range(2)]
                for h in range(nheads):
                    z = normalize(oT[h], h, lname)
                    kc, hh = divmod(h, 2)
                    tneg = wpool.tile([O, IB], F32, tag="tneg", bufs=2,
                                      name=f"tn_{lname}_{h}")
                    nc.vector.tensor_scalar_min(tneg[:], z[:], 0.0)
                    eneg = wpool.tile([O, IB], F32, tag="eneg", bufs=2,
                                      name=f"en_{lname}_{h}")
                    nc.scalar.activation(eneg[:], tneg[:], AF.Exp)
                    rpos = wpool.tile([O, IB], F32, tag="rpos", bufs=2,
                                      name=f"rp_{lname}_{h}")
                    nc.vector.tensor_scalar(rpos[:], z[:], 0.0, -1.0,
                                            ALU.max, ALU.add)
                    nc.vector.tensor_add(
                        xTn[kc][hh * O:(hh + 1) * O, :], eneg[:], rpos[:])

                # next-layer local features h = xTn @ W  (per 128-row chunk)
                for ic in range(4):
                    phn = pp.tile([128, next_heads * O], F32, tag="work",
                                  name=f"phn_{lname}_{ic}")
                    for h in range(next_heads):
                        for kc in range(2):
                            nc.tensor.matmul(
                                phn[:, h * O:(h + 1) * O],
                                xTn[kc][:, ic * 128:(ic + 1) * 128],
                                w_sb[:, (kc * next_heads + h) * O:
                                     (kc * next_heads + h) * O + O],
                                start=(kc == 0), stop=(kc == 1),
                            )
                    hl = wpool.tile([128, next_heads * O], BF16, tag="hl", bufs=3,
                                    name=f"hl_{lname}_{ic}")
                    nc.vector.tensor_copy(hl[:], phn[:])
                    _dma(nc, gh_in[ic * 128:(ic + 1) * 128, :], hl[:])

                # next-layer el/er from xTn
                pelr = pp.tile([2 * next_heads, IB], F32, tag="work",
                               name=f"pelr_{lname}")
                for kc in range(2):
                    nc.tensor.matmul(
                        pelr[:],
                        wlr_sb[:, kc * 2 * next_heads:(kc + 1) * 2 * next_heads],
                        xTn[kc][:],
                        start=(kc == 0), stop=(kc == 1),
                    )
                elr_st = wpool.tile([2 * next_heads, IB], F32, tag="elr_st",
                                    bufs=2, name=f"elrst_{lname}")
                nc.vector.tensor_copy(elr_st[:], pelr[:])
                _dma(nc, ger_in[:], elr_st[next_heads:2 * next_heads, :])

                nc.gpsimd.collective_compute(
                    "AllGather", ALU.bypass, replica_groups=rg,
                    ins=[gh_in[:]], outs=[gh[:]])
                nc.gpsimd.collective_compute(
                    "AllGather", ALU.bypass, replica_groups=rg,
                    ins=[ger_in[:]], outs=[ger[:]])

                # gathered h -> per-j-tile [h | ones-column] tiles
                nc.vector.memset(h_all_n[:], 1.0)
                for jt in range(NT):
                    dst = h_all_n[:, jt * next_heads * 65:(jt + 1) * next_heads * 65]
                    dst = dst.rearrange("p (h c) -> p h c", h=next_heads)[:, :, 0:O]
                    _dma(nc, dst,
                         gh[jt * 128:(jt + 1) * 128, :].rearrange(
                             "p (h o) -> p h o", h=next_heads))

                gv = ger[:].rearrange("(r g) i -> g r i", g=next_heads)

                def load_er(erp, h):
                    _dma(nc, erp[1:2, :].rearrange("p (r i) -> p r i", r=NC),
                         gv[h:h + 1])

                def load_el(elp, h):
                    _dma(nc, elp[0:1, :], elr_st[h:h + 1, :])

                return load_er, load_el

            # ================= layer 0 =================
            oT0 = attention(
                H, h_all0, "l0",
                lambda erp, h: _dma(nc, erp[1:2, :], er_stage0[H + h:H + h + 1, :]),
                lambda elp, h: _dma(nc, elp[0:1, :], el_stage0[h:h + 1, :]),
            )
            h_all1 = wpool.tile([128, NT * H * 65], BF16, tag="h_all", bufs=1)
            ld_er1, ld_el1 = transition(oT0, H, w1_sb, wlr1_sb, H, gh1_in, gh1,
                                        ger1_in, ger1, h_all1, "t0")

            # ================= layer 1 =================
            oT1 = attention(H, h_all1, "l1", ld_er1, ld_el1)
            h_all2 = wpool.tile([128, NT * 65], BF16, tag="h_all", bufs=1)
            ld_er2, ld_el2 = transition(oT1, H, w2_sb, wlr2_sb, 1, gh2_in, gh2,
                                        ger2_in, ger2, h_all2, "t1")

            # ================= layer 2 =================
            oT2 = attention(1, h_all2, "l2", ld_er2, ld_el2)
            zf = normalize(oT2[0], 0, "l2f")
            _dma(nc, y[:], zf[:])

    nc.compile()
    return nc


def _get_nc():
    if "nc" not in _CACHE:
        _CACHE["nc"] = _build()
    return _CACHE["nc"]


def kernel(x, adj, W0, a0, W1, a1, W2, a2, **_):
    x = np.asarray(x, np.float32)
    adj = np.asarray(adj)
    W0 = np.asarray(W0, np.float32)
    W1 = np.asarray(W1, np.float32)
    W2 = np.asarray(W2, np.float32)
    a0 = np.asarray(a0, np.float32)
    a1 = np.asarray(a1, np.float32)
    a2 = np.asarray(a2, np.float32)

    # host-side layout prep (no model math beyond folding W @ a)
    xTf = np.ascontiguousarray(x.T)
    adj_bf = (adj != 0).astype(ml_dtypes.bfloat16)

    def fold(W, a):
        o = W.shape[-1]
        wl = np.einsum("hdo,ho->dh", W, a[:, :o, 0])
        wr = np.einsum("hdo,ho->dh", W, a[:, o:, 0])
        return np.ascontiguousarray(
            np.concatenate([wl, wr], axis=1).astype(np.float32))

    common = {
        "xTf": xTf,
        "w0": W0, "w1": W1, "w2": W2,
        "wlr0": fold(W0, a0), "wlr1": fold(W1, a1), "wlr2": fold(W2, a2),
    }
    in_maps = []
    for d in range(NC):
        rows = slice(d * IB, (d + 1) * IB)
        maskT = np.ascontiguousarray(adj_bf[rows].T).reshape(NT, 128, IB)
        in_maps.append({
            **common,
            "xT0": np.ascontiguousarray(xTf[:, rows]),
            "maskT": maskT,
        })

    nc = _get_nc()
    _CACHE["in_maps"] = in_maps
    res = run_bass_kernel_spmd(nc, in_maps, core_ids=list(range(NC)))
    out = np.empty((N, O), np.float32)
    for d in range(NC):
        out[d * IB:(d + 1) * IB] = res.results[d]["y"].T
    return out
